# revision 13
# baseline (speedup 1.0000x reference)
"""nn_Block dense transformer block on 8 trn2 NeuronCores.

v2: bf16 compute, SBUF-resident K/V/Q (no DRAM spill), uniform causal
skip (8+16 s-tiles per q-chunk instead of 32), batched exp on the
scalar engine, fused DVE epilogues, and emission ordered so FFN/proj of
q-chunk 0 overlaps the scalar-engine-bound attention of q-chunk 1.

Sharding: core = (batch b = core//2, j = core%2). Each core handles
1024 query rows of its batch: chunk0 = rows [512j, 512j+512) (needs
s-tiles 0..8), chunk1 = rows [1024+512j, 1536+512j) (needs 0..16).
All cores run the identical instruction stream; causal masking is in
the (per-core) mask data.
"""

from contextlib import ExitStack

import numpy as np

import concourse.bass as bass
import concourse.mybir as mybir
import concourse.tile as tile

f32 = mybir.dt.float32
f32r = mybir.dt.float32r
f16 = mybir.dt.float16
bf16 = mybir.dt.bfloat16
AF = mybir.ActivationFunctionType
ALU = mybir.AluOpType

B, T, C, H, D = 4, 2048, 1024, 16, 64
QN = 1024           # query rows per core
NCT = C // 128      # 8 c-tiles
NST = T // 128      # 16 s-tiles
NFT = 4 * C // 128  # 32 f-tiles (FFN hidden)
NS = (8, 16)        # uniform causal s-tile counts for q-chunk 0 / 1
EPS = 1e-5
CH = 512


def split_excess_waits(nc, max_waits=1):
    """walrus accepts at most 1 sem-wait per instruction; move excess
    waits onto NoOps inserted before the instruction on the same engine."""
    n = 0
    for f in nc.m.functions:
        for blk in f.blocks:
            insts = blk.instructions
            i = 0
            while i < len(insts):
                inst = insts[i]
                si = inst.sync_info
                if si is not None and len(si.on_wait) > max_waits:
                    waits = list(si.on_wait)
                    excess, keep = waits[:-max_waits], waits[-max_waits:]
                    si.on_wait = keep
                    for w in excess:
                        n += 1
                        nop = mybir.InstNoOp(name=f"I-wsplit-{n}", ins=[], outs=[])
                        nop.engine = inst.engine
                        nop.sync_info = mybir.SyncInfo(on_wait=[w], on_update=[])
                        insts.insert(i, nop)
                        i += 1
                i += 1
    return n


def build_program():
    nc = bass.Bass()

    d = {}
    d["xT"] = nc.dram_tensor("xT", [C, T], bf16, kind="ExternalInput")
    d["xqT"] = nc.dram_tensor("xqT", [C, QN], bf16, kind="ExternalInput")
    d["wq_r"] = nc.dram_tensor("wq_r", [C, C], bf16, kind="ExternalInput")
    d["wk_r"] = nc.dram_tensor("wk_r", [C, C], bf16, kind="ExternalInput")
    d["wv_r"] = nc.dram_tensor("wv_r", [C, C], bf16, kind="ExternalInput")
    d["wp_t"] = nc.dram_tensor("wp_t", [NCT, 128, NCT, 128], bf16,
                               kind="ExternalInput")
    d["w1_t"] = nc.dram_tensor("w1_t", [NFT, 128, NCT, 128], bf16,
                               kind="ExternalInput")
    d["w2_t"] = nc.dram_tensor("w2_t", [NCT, 128, NFT, 128], bf16,
                               kind="ExternalInput")
    d["qb_in"] = nc.dram_tensor("qb_in", [128, NCT], f32, kind="ExternalInput")
    d["kb_in"] = nc.dram_tensor("kb_in", [128, NCT], f32, kind="ExternalInput")
    d["vb_rep"] = nc.dram_tensor("vb_rep", [128, C], bf16, kind="ExternalInput")
    d["bproj_in"] = nc.dram_tensor("bproj_in", [128, NCT], f32,
                                   kind="ExternalInput")
    d["b1_in"] = nc.dram_tensor("b1_in", [128, NFT], f32, kind="ExternalInput")
    d["b2_in"] = nc.dram_tensor("b2_in", [128, NCT], f32, kind="ExternalInput")
    d["t_rep"] = nc.dram_tensor("t_rep", [128, QN], f32, kind="ExternalInput")
    d["s_iota"] = nc.dram_tensor("s_iota", [128, NST], f32,
                                 kind="ExternalInput")
    d["ones_lhs"] = nc.dram_tensor("ones_lhs", [128, 1], bf16,
                                   kind="ExternalInput")
    d["ones_row"] = nc.dram_tensor("ones_row", [1, 128], bf16,
                                   kind="ExternalInput")
    d["ones_all"] = nc.dram_tensor("ones_all", [128, 128], bf16,
                                   kind="ExternalInput")
    d["outT"] = nc.dram_tensor("outT", [C, QN], bf16, kind="ExternalOutput")
    with tile.TileContext(nc) as tc:
        with nc.allow_low_precision(reason="bf16 compute is intended"):
            _build_body(nc, tc, d)
    return nc


def _build_body(nc, tc, d):
    with ExitStack() as top:
        singles = top.enter_context(tc.tile_pool(name="singles", bufs=1))
        ones_lhs = singles.tile([128, 1], bf16)
        nc.sync.dma_start(out=ones_lhs[:], in_=d["ones_lhs"].ap())
        ones_row = singles.tile([1, 128], bf16)
        nc.sync.dma_start(out=ones_row[:], in_=d["ones_row"].ap())
        ones_all = singles.tile([128, 128], bf16)
        nc.sync.dma_start(out=ones_all[:], in_=d["ones_all"].ap())
        qb_sb = singles.tile([128, NCT], f32)
        nc.sync.dma_start(out=qb_sb[:], in_=d["qb_in"].ap())
        kb_sb = singles.tile([128, NCT], f32)
        nc.sync.dma_start(out=kb_sb[:], in_=d["kb_in"].ap())
        vb_sb = singles.tile([128, C], bf16)
        nc.sync.dma_start(out=vb_sb[:], in_=d["vb_rep"].ap())
        bp_sb = singles.tile([128, NCT], f32)
        nc.sync.dma_start(out=bp_sb[:], in_=d["bproj_in"].ap())
        b1_sb = singles.tile([128, NFT], f32)
        nc.sync.dma_start(out=b1_sb[:], in_=d["b1_in"].ap())
        b2_sb = singles.tile([128, NCT], f32)
        nc.sync.dma_start(out=b2_sb[:], in_=d["b2_in"].ap())
        t_rep = singles.tile([128, QN], f32)
        nc.sync.dma_start(out=t_rep[:], in_=d["t_rep"].ap())
        s_iota = singles.tile([128, NST], f32)
        nc.sync.dma_start(out=s_iota[:], in_=d["s_iota"].ap())

        # persistent activation tensors
        eps_sb = singles.tile([1, 1], f32)
        nc.vector.memset(eps_sb[:], EPS)
        scm1 = singles.tile([128, 1], f32)
        nc.vector.memset(scm1[:], -1.0)
        scmh = singles.tile([128, 1], f32)
        nc.vector.memset(scmh[:], -0.5)

        kt = singles.tile([128, NCT, T], bf16)       # K^T (2 heads x 64d rows)
        v = singles.tile([128, NCT, NST, 2, 65], bf16)  # V rows=s-in-st
        qT = singles.tile([128, NCT, QN], bf16)
        for hp in range(NCT):
            nc.vector.memset(v[:, hp, :, :, 64:65], 1.0)

        lnc = (ones_lhs, ones_row, eps_sb, scmh)

        # ---------------- phase A: LN1 + QKV projections ----------------
        with ExitStack() as pA:
            xn = None
            bigA = pA.enter_context(tc.tile_pool(name="bigA", bufs=1))
            xn = bigA.tile([128, NCT, T], bf16)
            xnq = bigA.tile([128, NCT, QN], bf16)
            wstream = pA.enter_context(tc.tile_pool(name="wstream", bufs=2))
            xchunk = pA.enter_context(tc.tile_pool(name="xchunk", bufs=2))
            sqp = pA.enter_context(tc.tile_pool(name="lnsq", bufs=2))
            rows = pA.enter_context(tc.tile_pool(name="lnrows", bufs=1))
            absb = pA.enter_context(tc.tile_pool(name="lnabsb", bufs=2))
            stat_ps = pA.enter_context(
                tc.tile_pool(name="statps", bufs=1, space="PSUM"))
            rep_ps = pA.enter_context(
                tc.tile_pool(name="repps", bufs=1, space="PSUM"))
            mm_ps = pA.enter_context(
                tc.tile_pool(name="mmps", bufs=4, space="PSUM"))
            lnp = (sqp, rows, absb, stat_ps, rep_ps)

            # LN1 on context (into xn) and on the q-columns (into xnq).
            # Stream x in chunks through xchunk.
            for ch in range(T // CH):
                xc = xchunk.tile([128, NCT, CH], bf16, tag="xc")
                nc.sync.dma_start(
                    out=xc[:],
                    in_=d["xT"].ap().rearrange("(ko p) t -> p ko t", p=128)
                    [:, :, ch * CH:(ch + 1) * CH])
                _ln_chunk(nc, lnp, xc, xn, ch * CH, lnc)
            for ch in range(QN // CH):
                xc = xchunk.tile([128, NCT, CH], bf16, tag="xc")
                nc.sync.dma_start(
                    out=xc[:],
                    in_=d["xqT"].ap().rearrange("(ko p) t -> p ko t", p=128)
                    [:, :, ch * CH:(ch + 1) * CH])
                _ln_chunk(nc, lnp, xc, xnq, ch * CH, lnc)

            # --- Q^T --- (weights streamed in column halves)
            for wh in range(2):
                wsb = wstream.tile([128, NCT, CH], bf16, tag="w")
                nc.sync.dma_start(
                    out=wsb[:],
                    in_=d["wq_r"].ap().rearrange("(ko p) n -> p ko n", p=128)
                    [:, :, wh * CH:(wh + 1) * CH])
                for ml in range(4):
                    m = 4 * wh + ml
                    for qc in range(QN // CH):
                        sl = slice(qc * CH, (qc + 1) * CH)
                        ps = mm_ps.tile([128, CH], f32, tag="mm")
                        for ct in range(NCT):
                            nc.tensor.matmul(
                                ps[:], wsb[:, ct, ml * 128:(ml + 1) * 128],
                                xnq[:, ct, sl],
                                start=(ct == 0), stop=(ct == NCT - 1))
                        nc.vector.tensor_scalar(out=qT[:, m, sl], in0=ps[:],
                                                scalar1=qb_sb[:, m:m + 1],
                                                scalar2=None, op0=ALU.add)
            # --- K^T ---
            for wh in range(2):
                wsb = wstream.tile([128, NCT, CH], bf16, tag="w")
                nc.sync.dma_start(
                    out=wsb[:],
                    in_=d["wk_r"].ap().rearrange("(ko p) n -> p ko n", p=128)
                    [:, :, wh * CH:(wh + 1) * CH])
                for ml in range(4):
                    m = 4 * wh + ml
                    for sc in range(T // CH):
                        sl = slice(sc * CH, (sc + 1) * CH)
                        ps = mm_ps.tile([128, CH], f32, tag="mm")
                        for ct in range(NCT):
                            nc.tensor.matmul(
                                ps[:], wsb[:, ct, ml * 128:(ml + 1) * 128],
                                xn[:, ct, sl],
                                start=(ct == 0), stop=(ct == NCT - 1))
                        nc.vector.tensor_scalar(out=kt[:, m, sl], in0=ps[:],
                                                scalar1=kb_sb[:, m:m + 1],
                                                scalar2=None, op0=ALU.add)
            # --- V ---
            for half in range(2):
                wsb = wstream.tile([128, NCT, CH], bf16, tag="w")
                nc.sync.dma_start(
                    out=wsb[:],
                    in_=d["wv_r"].ap().rearrange("(ko p) n -> p ko n", p=128)
                    [:, :, half * CH:(half + 1) * CH])
                for st in range(NST):
                    ps = mm_ps.tile([128, CH], f32, tag="mm")
                    for ct in range(NCT):
                        nc.tensor.matmul(ps[:],
                                         xn[:, ct, st * 128:(st + 1) * 128],
                                         wsb[:, ct, :],
                                         start=(ct == 0), stop=(ct == NCT - 1))
                    psv = ps[:].rearrange("p (a x) -> p a x", a=4)
                    vbv = vb_sb[:].rearrange("p (hh a x) -> p hh a x", hh=2, a=4)
                    for b in range(2):
                        nc.vector.scalar_tensor_tensor(
                            out=v[:, 4 * half:4 * half + 4, st, b, 0:64],
                            in0=psv[:, :, b * 64:(b + 1) * 64],
                            scalar=0.0,
                            in1=vbv[:, half, :, b * 64:(b + 1) * 64],
                            op0=ALU.add, op1=ALU.add)

        # ---------------- phase B: attention + proj + LN2 + FFN ----------
        with ExitStack() as pB:
            bigB = pB.enter_context(tc.tile_pool(name="bigB", bufs=1))
            attnT = bigB.tile([128, NCT, QN], bf16)
            x2 = bigB.tile([128, NCT, QN], bf16)
            xn2 = bigB.tile([128, NCT, QN], bf16)
            hT = bigB.tile([128, NFT, CH], bf16)
            weip = pB.enter_context(tc.tile_pool(name="weip", bufs=2))
            maskp = pB.enter_context(tc.tile_pool(name="maskp", bufs=2))
            normp = pB.enter_context(tc.tile_pool(name="normp", bufs=2))
            w12p = pB.enter_context(tc.tile_pool(name="w12p", bufs=2))
            otp = pB.enter_context(tc.tile_pool(name="otp", bufs=1))
            sq2 = pB.enter_context(tc.tile_pool(name="lnsq2", bufs=1))
            rows2 = pB.enter_context(tc.tile_pool(name="lnrows2", bufs=1))
            absb2 = pB.enter_context(tc.tile_pool(name="lnabsb2", bufs=1))
            sc_ps = pB.enter_context(
                tc.tile_pool(name="scps", bufs=1, space="PSUM"))
            av_ps = pB.enter_context(
                tc.tile_pool(name="avps", bufs=1, space="PSUM"))
            mm2_ps = pB.enter_context(
                tc.tile_pool(name="mm2ps", bufs=2, space="PSUM"))
            ln2p = (sq2, rows2, absb2, mm2_ps, mm2_ps)

            # residual base: x2 starts as x (q columns)
            nc.sync.dma_start(
                out=x2[:],
                in_=d["xqT"].ap().rearrange("(ko p) t -> p ko t", p=128))

            def attention(qc):
                nst = NS[qc]
                qsl = slice(qc * CH, (qc + 1) * CH)
                for hp in range(NCT):
                    psA = av_ps.tile([128, CH], f32, tag="avA")
                    psB = av_ps.tile([128, CH], f32, tag="avB")
                    for st in range(nst):
                        ssl = slice(st * 128, (st + 1) * 128)
                        scA = sc_ps.tile([128, CH], f32, tag="scA", bufs=2)
                        scB = sc_ps.tile([128, CH], f32, tag="scB", bufs=2)
                        nc.tensor.matmul(scA[:], kt[0:64, hp, ssl],
                                         qT[0:64, hp, qsl],
                                         start=True, stop=True,
                                         tile_position=(0, 0))
                        nc.tensor.matmul(scB[:], kt[64:128, hp, ssl],
                                         qT[64:128, hp, qsl],
                                         start=True, stop=True,
                                         tile_position=(64, 0))
                        weiA = weip.tile([128, CH], bf16, tag="weiA", bufs=3)
                        weiB = weip.tile([128, CH], bf16, tag="weiB", bufs=3)
                        nc.scalar.activation(out=weiA[:], in_=scA[:], func=AF.Exp)
                        nc.scalar.activation(out=weiB[:], in_=scB[:], func=AF.Exp)
                        if qc == 0 or st >= NS[0]:
                            mk = maskp.tile([128, CH], bf16, tag="mk")
                            nc.vector.tensor_scalar(
                                out=mk[:], in0=t_rep[:, qsl],
                                scalar1=s_iota[:, st:st + 1], scalar2=None,
                                op0=ALU.is_ge)
                            nc.vector.tensor_mul(weiA[:], weiA[:], mk[:])
                            nc.vector.tensor_mul(weiB[:], weiB[:], mk[:])
                        nc.tensor.matmul(psA[0:65, :], v[:, hp, st, 0, :],
                                         weiA[:],
                                         start=(st == 0), stop=(st == nst - 1))
                        nc.tensor.matmul(psB[0:65, :], v[:, hp, st, 1, :],
                                         weiB[:],
                                         start=(st == 0), stop=(st == nst - 1))
                    lnA = normp.tile([65, CH], f32, tag="lnA", bufs=1)
                    lnB = normp.tile([65, CH], f32, tag="lnB", bufs=1)
                    nc.scalar.activation(out=lnA[64:65, :], in_=psA[64:65, :],
                                         func=AF.Ln)
                    nc.scalar.activation(out=lnB[64:65, :], in_=psB[64:65, :],
                                         func=AF.Ln)
                    recA = normp.tile([65, CH], bf16, tag="recA", bufs=1)
                    recB = normp.tile([65, CH], bf16, tag="recB", bufs=1)
                    nc.scalar.activation(out=recA[64:65, :], in_=lnA[64:65, :],
                                         func=AF.Exp, scale=scm1[64:65, :])
                    nc.scalar.activation(out=recB[64:65, :], in_=lnB[64:65, :],
                                         func=AF.Exp, scale=scm1[64:65, :])
                    repA = sc_ps.tile([128, CH], f32, tag="scA", bufs=2)
                    repB = sc_ps.tile([128, CH], f32, tag="scB", bufs=2)
                    nc.tensor.matmul(repA[:], ones_all[64:65, :],
                                     recA[64:65, :],
                                     start=True, stop=True)
                    nc.tensor.matmul(repB[:], ones_all[64:65, :],
                                     recB[64:65, :],
                                     start=True, stop=True)
                    rsbA = normp.tile([128, CH], f32, tag="rsbA", bufs=1)
                    rsbB = normp.tile([128, CH], f32, tag="rsbB", bufs=1)
                    nc.vector.tensor_copy(rsbA[:], repA[:])
                    nc.vector.tensor_copy(rsbB[:], repB[:])
                    nc.vector.tensor_mul(attnT[0:64, hp, qsl],
                                         psA[0:64, :], rsbA[0:64, :])
                    tmpB = normp.tile([64, CH], bf16, tag="tmpB")
                    nc.vector.tensor_mul(tmpB[:], psB[0:64, :], rsbB[0:64, :])
                    nc.sync.dma_start(out=attnT[64:128, hp, qsl], in_=tmpB[:])

            def proj(qc):
                qsl = slice(qc * CH, (qc + 1) * CH)
                for ct in range(NCT):
                    wp = w12p.tile([128, NCT, 128], bf16, tag="w1")
                    nc.sync.dma_start(out=wp[:], in_=d["wp_t"].ap()[ct])
                    ps = mm2_ps.tile([128, CH], f32, tag="mm")
                    for m in range(NCT):
                        nc.tensor.matmul(ps[:], wp[:, m, :], attnT[:, m, qsl],
                                         start=(m == 0), stop=(m == NCT - 1))
                    nc.vector.scalar_tensor_tensor(
                        out=x2[:, ct, qsl], in0=ps[:],
                        scalar=bp_sb[:, ct:ct + 1], in1=x2[:, ct, qsl],
                        op0=ALU.add, op1=ALU.add)

            def ffn(qc):
                qsl = slice(qc * CH, (qc + 1) * CH)
                for ft in range(NFT):
                    w1s = w12p.tile([128, NCT, 128], bf16, tag="w1")
                    nc.sync.dma_start(out=w1s[:], in_=d["w1_t"].ap()[ft])
                    ps = mm2_ps.tile([128, CH], f32, tag="mm")
                    for ct in range(NCT):
                        nc.tensor.matmul(ps[:], w1s[:, ct, :], xn2[:, ct, qsl],
                                         start=(ct == 0), stop=(ct == NCT - 1))
                    nc.vector.tensor_scalar(out=hT[:, ft, :], in0=ps[:],
                                            scalar1=b1_sb[:, ft:ft + 1],
                                            scalar2=0.0, op0=ALU.add,
                                            op1=ALU.max)
                for ct in range(NCT):
                    ps = mm2_ps.tile([128, CH], f32, tag="mm")
                    for g in range(4):
                        w2s = w12p.tile([128, 8, 128], bf16, tag="w2")
                        nc.sync.dma_start(
                            out=w2s[:],
                            in_=d["w2_t"].ap()[ct][:, g * 8:(g + 1) * 8, :])
                        for f in range(8):
                            ft = g * 8 + f
                            nc.tensor.matmul(ps[:], w2s[:, f, :], hT[:, ft, :],
                                             start=(ft == 0),
                                             stop=(ft == NFT - 1))
                    ot = otp.tile([128, CH], bf16, tag="ot")
                    nc.vector.scalar_tensor_tensor(
                        out=ot[:], in0=ps[:], scalar=b2_sb[:, ct:ct + 1],
                        in1=x2[:, ct, qsl], op0=ALU.add, op1=ALU.add)
                    nc.sync.dma_start(
                        out=d["outT"].ap().rearrange(
                            "(ko p) t -> p ko t", p=128)[:, ct, qsl],
                        in_=ot[:])

            attention(0)
            proj(0)
            _ln_chunk2(nc, ln2p, x2, xn2, 0, lnc)
            attention(1)
            ffn(0)
            proj(1)
            _ln_chunk2(nc, ln2p, x2, xn2, CH, lnc)
            ffn(1)


def _ln_chunk(nc, pools, xc, dst_sb, col0, consts):
    """LN over one CH-column chunk; xc is [128, NCT, CH] (chunk of src),
    dst_sb is the full [128, NCT, *] output tile."""
    sqp, rows, absb, stat_ps, rep_ps = pools
    ones_lhs, ones_row, eps, scmh = consts
    sl = slice(col0, col0 + CH)
    ps_sum = stat_ps.tile([1, CH], f32, tag="ssum")
    ps_sq = stat_ps.tile([1, CH], f32, tag="ssq")
    for ct in range(NCT):
        sq = sqp.tile([128, CH], bf16, tag="ln_sq")
        nc.vector.tensor_mul(sq[:], xc[:, ct, :], xc[:, ct, :])
        nc.tensor.matmul(ps_sum[:], ones_lhs[:], xc[:, ct, :],
                         start=(ct == 0), stop=(ct == NCT - 1))
        nc.tensor.matmul(ps_sq[:], ones_lhs[:], sq[:],
                         start=(ct == 0), stop=(ct == NCT - 1))
    mean = rows.tile([1, CH], f32, tag="ln_mean")
    nc.vector.tensor_scalar_mul(mean[:], ps_sum[:], 1.0 / C)
    var = rows.tile([1, CH], f32, tag="ln_var")
    nc.vector.tensor_mul(var[:], mean[:], mean[:])
    nc.vector.scalar_tensor_tensor(out=var[:], in0=ps_sq[:], scalar=1.0 / C,
                                   in1=var[:], op0=ALU.mult, op1=ALU.subtract)
    lnv = rows.tile([1, CH], f32, tag="ln_lnv")
    nc.scalar.activation(out=lnv[:], in_=var[:], func=AF.Ln,
                         bias=eps[0:1, :], scale=1.0)
    arow = rows.tile([1, CH], bf16, tag="ln_a")
    nc.scalar.activation(out=arow[:], in_=lnv[:], func=AF.Exp,
                         scale=scmh[0:1, :])
    brow = rows.tile([1, CH], bf16, tag="ln_b")
    nc.vector.scalar_tensor_tensor(out=brow[:], in0=mean[:], scalar=-1.0,
                                   in1=arow[:], op0=ALU.mult, op1=ALU.mult)
    ps_a = rep_ps.tile([128, CH], f32, tag="ra")
    ps_b = rep_ps.tile([128, CH], f32, tag="rb")
    nc.tensor.matmul(ps_a[:], ones_row[:], arow[:], start=True, stop=True)
    nc.tensor.matmul(ps_b[:], ones_row[:], brow[:], start=True, stop=True)
    a_sb = absb.tile([128, CH], bf16, tag="ln_asb")
    b_sb = absb.tile([128, CH], bf16, tag="ln_bsb")
    nc.vector.tensor_copy(a_sb[:], ps_a[:])
    nc.vector.tensor_copy(b_sb[:], ps_b[:])
    for ct in range(NCT):
        nc.vector.tensor_mul(dst_sb[:, ct, sl], xc[:, ct, :], a_sb[:])
        nc.vector.tensor_add(dst_sb[:, ct, sl], dst_sb[:, ct, sl], b_sb[:])


def _ln_chunk2(nc, pools, src_sb, dst_sb, col0, consts):
    """LN over one CH-column chunk of a resident [128, NCT, QN] tile."""
    sqp, rows, absb, stat_ps, rep_ps = pools
    ones_lhs, ones_row, eps, scmh = consts
    sl = slice(col0, col0 + CH)
    ps_sum = stat_ps.tile([1, CH], f32, tag="mm")
    ps_sq = stat_ps.tile([1, CH], f32, tag="mm")
    for ct in range(NCT):
        sq = sqp.tile([128, CH], bf16, tag="ln_sq2")
        nc.vector.tensor_mul(sq[:], src_sb[:, ct, sl], src_sb[:, ct, sl])
        nc.tensor.matmul(ps_sum[:], ones_lhs[:], src_sb[:, ct, sl],
                         start=(ct == 0), stop=(ct == NCT - 1))
        nc.tensor.matmul(ps_sq[:], ones_lhs[:], sq[:],
                         start=(ct == 0), stop=(ct == NCT - 1))
    mean = rows.tile([1, CH], f32, tag="ln_mean2")
    nc.vector.tensor_scalar_mul(mean[:], ps_sum[:], 1.0 / C)
    var = rows.tile([1, CH], f32, tag="ln_var2")
    nc.vector.tensor_mul(var[:], mean[:], mean[:])
    nc.vector.scalar_tensor_tensor(out=var[:], in0=ps_sq[:], scalar=1.0 / C,
                                   in1=var[:], op0=ALU.mult, op1=ALU.subtract)
    lnv = rows.tile([1, CH], f32, tag="ln_lnv2")
    nc.scalar.activation(out=lnv[:], in_=var[:], func=AF.Ln,
                         bias=eps[0:1, :], scale=1.0)
    arow = rows.tile([1, CH], bf16, tag="ln_a2")
    nc.scalar.activation(out=arow[:], in_=lnv[:], func=AF.Exp,
                         scale=scmh[0:1, :])
    brow = rows.tile([1, CH], bf16, tag="ln_b2")
    nc.vector.scalar_tensor_tensor(out=brow[:], in0=mean[:], scalar=-1.0,
                                   in1=arow[:], op0=ALU.mult, op1=ALU.mult)
    ps_a = rep_ps.tile([128, CH], f32, tag="mm")
    nc.tensor.matmul(ps_a[:], ones_row[:], arow[:], start=True, stop=True)
    a_sb = absb.tile([128, CH], bf16, tag="ln_asb2")
    nc.vector.tensor_copy(a_sb[:], ps_a[:])
    ps_b = rep_ps.tile([128, CH], f32, tag="mm")
    nc.tensor.matmul(ps_b[:], ones_row[:], brow[:], start=True, stop=True)
    b_sb = absb.tile([128, CH], bf16, tag="ln_bsb2")
    nc.vector.tensor_copy(b_sb[:], ps_b[:])
    for ct in range(NCT):
        nc.vector.tensor_mul(dst_sb[:, ct, sl], src_sb[:, ct, sl], a_sb[:])
        nc.vector.tensor_add(dst_sb[:, ct, sl], dst_sb[:, ct, sl], b_sb[:])


def make_host_inputs(inputs):
    """Build per-core in_maps from the full problem inputs."""
    import ml_dtypes
    bfl = ml_dtypes.bfloat16

    x = np.asarray(inputs["x"], np.float32)
    wq = np.asarray(inputs["wq"], np.float32)
    wk = np.asarray(inputs["wk"], np.float32)
    wv = np.asarray(inputs["wv"], np.float32)
    w_proj = np.asarray(inputs["w_proj"], np.float32)
    b_proj = np.asarray(inputs["b_proj"], np.float32)
    w1 = np.asarray(inputs["w1"], np.float32)
    b1 = np.asarray(inputs["b1"], np.float32)
    w2 = np.asarray(inputs["w2"], np.float32)
    b2 = np.asarray(inputs["b2"], np.float32)
    g1 = np.asarray(inputs["g1"], np.float32)
    be1 = np.asarray(inputs["be1"], np.float32)
    g2 = np.asarray(inputs["g2"], np.float32)
    be2 = np.asarray(inputs["be2"], np.float32)

    scale = np.float32(C ** -0.5)
    wq_eff = wq * g1[None, :, None]
    wk_eff = wk * g1[None, :, None]
    wv_eff = wv * g1[None, :, None]
    wq_r = np.ascontiguousarray(
        wq_eff.transpose(1, 0, 2).reshape(C, C) * scale).astype(bfl)
    wk_r = np.ascontiguousarray(
        wk_eff.transpose(1, 0, 2).reshape(C, C)).astype(bfl)
    wv_r = np.ascontiguousarray(
        wv_eff.transpose(1, 0, 2).reshape(C, C)).astype(bfl)
    qb = (np.einsum("c,hcd->hd", be1, wq_eff).reshape(C) * scale)
    kb = np.einsum("c,hcd->hd", be1, wk_eff).reshape(C)
    vb = np.einsum("c,hcd->hd", be1, wv_eff).reshape(C)
    wp_t = np.ascontiguousarray(
        w_proj.reshape(NCT, 128, NCT, 128).transpose(2, 1, 0, 3)).astype(bfl)
    w1_eff = w1 * g2[:, None]
    b1_eff = b1 + be2 @ w1
    w1_t = np.ascontiguousarray(
        w1_eff.reshape(NCT, 128, NFT, 128).transpose(2, 1, 0, 3)).astype(bfl)
    w2_t = np.ascontiguousarray(
        w2.reshape(NFT, 128, NCT, 128).transpose(2, 1, 0, 3)).astype(bfl)

    shared = {
        "wq_r": wq_r, "wk_r": wk_r, "wv_r": wv_r,
        "wp_t": wp_t, "w1_t": w1_t, "w2_t": w2_t,
        "qb_in": np.ascontiguousarray(qb.reshape(NCT, 128).T),
        "kb_in": np.ascontiguousarray(kb.reshape(NCT, 128).T),
        "vb_rep": np.broadcast_to(vb[None, :], (128, C)).astype(bfl).copy(),
        "bproj_in": np.ascontiguousarray(b_proj.reshape(NCT, 128).T),
        "b1_in": np.ascontiguousarray(b1_eff.reshape(NFT, 128).T),
        "b2_in": np.ascontiguousarray(b2.reshape(NCT, 128).T),
        "ones_lhs": np.ones((128, 1), bfl),
        "ones_row": np.ones((1, 128), bfl),
        "ones_all": np.ones((128, 128), bfl),
        "s_iota": np.ascontiguousarray(
            np.arange(T, dtype=np.float32).reshape(NST, 128).T),
    }

    in_maps = []
    qrows_all = []
    xT_cache = {}
    for core in range(8):
        b = core // 2
        j = core % 2
        qrows = np.r_[512 * j:512 * j + 512, 1024 + 512 * j:1536 + 512 * j]
        qrows_all.append((b, qrows))
        if b not in xT_cache:
            xT_cache[b] = np.ascontiguousarray(x[b].T).astype(bfl)
        m = dict(shared)
        m["xT"] = xT_cache[b]
        m["xqT"] = np.ascontiguousarray(x[b][qrows].T).astype(bfl)
        m["t_rep"] = np.broadcast_to(
            qrows.astype(np.float32)[None, :], (128, QN)).copy()
        in_maps.append(m)
    return in_maps, qrows_all


def gather_output(results, qrows_all):
    out = np.zeros((B, T, C), np.float32)
    for core, (b, qrows) in enumerate(qrows_all):
        out[b, qrows, :] = np.asarray(results[core]["outT"],
                                      dtype=np.float32).T
    return out


_NC_CACHE = {}


def kernel(**inputs):
    from concourse.bass_utils import run_bass_kernel_spmd

    if "nc" not in _NC_CACHE:
        nc = build_program()
        split_excess_waits(nc)
        _NC_CACHE["nc"] = nc
    nc = _NC_CACHE["nc"]
    in_maps, qrows_all = make_host_inputs(inputs)
    res = run_bass_kernel_spmd(nc, in_maps, core_ids=list(range(8)))
    return gather_output(res.results, qrows_all)


# revision 14
# speedup vs baseline: 1.2235x; 1.2235x over previous
"""nn_Block dense transformer block on 8 trn2 NeuronCores.

v2: bf16 compute, SBUF-resident K/V/Q (no DRAM spill), uniform causal
skip (8+16 s-tiles per q-chunk instead of 32), batched exp on the
scalar engine, fused DVE epilogues, and emission ordered so FFN/proj of
q-chunk 0 overlaps the scalar-engine-bound attention of q-chunk 1.

Sharding: core = (batch b = core//2, j = core%2). Each core handles
1024 query rows of its batch: chunk0 = rows [512j, 512j+512) (needs
s-tiles 0..8), chunk1 = rows [1024+512j, 1536+512j) (needs 0..16).
All cores run the identical instruction stream; causal masking is in
the (per-core) mask data.
"""

from contextlib import ExitStack

import numpy as np

import concourse.bass as bass
import concourse.mybir as mybir
import concourse.tile as tile

f32 = mybir.dt.float32
f32r = mybir.dt.float32r
f16 = mybir.dt.float16
bf16 = mybir.dt.bfloat16
AF = mybir.ActivationFunctionType
ALU = mybir.AluOpType

B, T, C, H, D = 4, 2048, 1024, 16, 64
QN = 1024           # query rows per core
NCT = C // 128      # 8 c-tiles
NST = T // 128      # 16 s-tiles
NFT = 4 * C // 128  # 32 f-tiles (FFN hidden)
NS = (8, 16)        # uniform causal s-tile counts for q-chunk 0 / 1
EPS = 1e-5
CH = 512


def split_excess_waits(nc, max_waits=1):
    """walrus accepts at most 1 sem-wait per instruction; move excess
    waits onto NoOps inserted before the instruction on the same engine."""
    n = 0
    for f in nc.m.functions:
        for blk in f.blocks:
            insts = blk.instructions
            i = 0
            while i < len(insts):
                inst = insts[i]
                si = inst.sync_info
                if si is not None and len(si.on_wait) > max_waits:
                    waits = list(si.on_wait)
                    excess, keep = waits[:-max_waits], waits[-max_waits:]
                    si.on_wait = keep
                    for w in excess:
                        n += 1
                        nop = mybir.InstNoOp(name=f"I-wsplit-{n}", ins=[], outs=[])
                        nop.engine = inst.engine
                        nop.sync_info = mybir.SyncInfo(on_wait=[w], on_update=[])
                        insts.insert(i, nop)
                        i += 1
                i += 1
    return n


def build_program():
    nc = bass.Bass()

    d = {}
    d["xT"] = nc.dram_tensor("xT", [C, T], bf16, kind="ExternalInput")
    d["xqT"] = nc.dram_tensor("xqT", [C, QN], bf16, kind="ExternalInput")
    d["wq_r"] = nc.dram_tensor("wq_r", [C, C], bf16, kind="ExternalInput")
    d["wk_r"] = nc.dram_tensor("wk_r", [C, C], bf16, kind="ExternalInput")
    d["wv_r"] = nc.dram_tensor("wv_r", [C, C], bf16, kind="ExternalInput")
    d["wp_t"] = nc.dram_tensor("wp_t", [NCT, 128, NCT, 128], bf16,
                               kind="ExternalInput")
    d["w1_t"] = nc.dram_tensor("w1_t", [NFT, 128, NCT, 128], bf16,
                               kind="ExternalInput")
    d["w2_t"] = nc.dram_tensor("w2_t", [NCT, 128, NFT, 128], bf16,
                               kind="ExternalInput")
    d["qb_in"] = nc.dram_tensor("qb_in", [128, NCT], f32, kind="ExternalInput")
    d["kb_in"] = nc.dram_tensor("kb_in", [128, NCT], f32, kind="ExternalInput")
    d["vb_rep"] = nc.dram_tensor("vb_rep", [128, C], bf16, kind="ExternalInput")
    d["bproj_in"] = nc.dram_tensor("bproj_in", [128, NCT], f32,
                                   kind="ExternalInput")
    d["b1_in"] = nc.dram_tensor("b1_in", [128, NFT], f32, kind="ExternalInput")
    d["b2_in"] = nc.dram_tensor("b2_in", [128, NCT], f32, kind="ExternalInput")
    d["t_rep"] = nc.dram_tensor("t_rep", [128, QN], f32, kind="ExternalInput")
    d["s_iota"] = nc.dram_tensor("s_iota", [128, NST], f32,
                                 kind="ExternalInput")
    d["ones_lhs"] = nc.dram_tensor("ones_lhs", [128, 1], bf16,
                                   kind="ExternalInput")
    d["ones_row"] = nc.dram_tensor("ones_row", [1, 128], bf16,
                                   kind="ExternalInput")
    d["ones_all"] = nc.dram_tensor("ones_all", [128, 128], bf16,
                                   kind="ExternalInput")
    d["outT"] = nc.dram_tensor("outT", [C, QN], bf16, kind="ExternalOutput")
    with tile.TileContext(nc) as tc:
        with nc.allow_low_precision(reason="bf16 compute is intended"):
            _build_body(nc, tc, d)
    return nc


def _build_body(nc, tc, d):
    with ExitStack() as top:
        singles = top.enter_context(tc.tile_pool(name="singles", bufs=1))
        ones_lhs = singles.tile([128, 1], bf16)
        nc.sync.dma_start(out=ones_lhs[:], in_=d["ones_lhs"].ap())
        ones_row = singles.tile([1, 128], bf16)
        nc.sync.dma_start(out=ones_row[:], in_=d["ones_row"].ap())
        ones_all = singles.tile([128, 128], bf16)
        nc.sync.dma_start(out=ones_all[:], in_=d["ones_all"].ap())
        qb_sb = singles.tile([128, NCT], f32)
        nc.sync.dma_start(out=qb_sb[:], in_=d["qb_in"].ap())
        kb_sb = singles.tile([128, NCT], f32)
        nc.sync.dma_start(out=kb_sb[:], in_=d["kb_in"].ap())
        vb_sb = singles.tile([128, C], bf16)
        nc.sync.dma_start(out=vb_sb[:], in_=d["vb_rep"].ap())
        bp_sb = singles.tile([128, NCT], f32)
        nc.sync.dma_start(out=bp_sb[:], in_=d["bproj_in"].ap())
        b1_sb = singles.tile([128, NFT], f32)
        nc.sync.dma_start(out=b1_sb[:], in_=d["b1_in"].ap())
        b2_sb = singles.tile([128, NCT], f32)
        nc.sync.dma_start(out=b2_sb[:], in_=d["b2_in"].ap())
        t_rep = singles.tile([128, QN], f32)
        nc.sync.dma_start(out=t_rep[:], in_=d["t_rep"].ap())
        s_iota = singles.tile([128, NST], f32)
        nc.sync.dma_start(out=s_iota[:], in_=d["s_iota"].ap())

        # persistent activation tensors
        eps_sb = singles.tile([1, 1], f32)
        nc.vector.memset(eps_sb[:], EPS)
        scm1 = singles.tile([128, 1], f32)
        nc.vector.memset(scm1[:], -1.0)
        scmh = singles.tile([128, 1], f32)
        nc.vector.memset(scmh[:], -0.5)

        kt = singles.tile([128, NCT, T], bf16)       # K^T (2 heads x 64d rows)
        v = singles.tile([128, NCT, NST, 2, 65], bf16)  # V rows=s-in-st
        qT = singles.tile([128, NCT, QN], bf16)
        for hp in range(NCT):
            nc.vector.memset(v[:, hp, :, :, 64:65], 1.0)

        lnc = (ones_lhs, ones_row, eps_sb, scmh)

        # ---------------- phase A: LN1 + QKV projections ----------------
        with ExitStack() as pA:
            xn = None
            bigA = pA.enter_context(tc.tile_pool(name="bigA", bufs=1))
            xn = bigA.tile([128, NCT, T], bf16)
            xnq = bigA.tile([128, NCT, QN], bf16)
            wstream = pA.enter_context(tc.tile_pool(name="wstream", bufs=2))
            xchunk = pA.enter_context(tc.tile_pool(name="xchunk", bufs=2))
            sqp = pA.enter_context(tc.tile_pool(name="lnsq", bufs=2))
            rows = pA.enter_context(tc.tile_pool(name="lnrows", bufs=1))
            absb = pA.enter_context(tc.tile_pool(name="lnabsb", bufs=2))
            stat_ps = pA.enter_context(
                tc.tile_pool(name="statps", bufs=1, space="PSUM"))
            rep_ps = pA.enter_context(
                tc.tile_pool(name="repps", bufs=1, space="PSUM"))
            mm_ps = pA.enter_context(
                tc.tile_pool(name="mmps", bufs=4, space="PSUM"))
            lnp = (sqp, rows, absb, stat_ps, rep_ps)

            # LN1 on context (into xn) and on the q-columns (into xnq).
            # Stream x in chunks through xchunk.
            for ch in range(T // CH):
                xc = xchunk.tile([128, NCT, CH], bf16, tag="xc")
                nc.sync.dma_start(
                    out=xc[:],
                    in_=d["xT"].ap().rearrange("(ko p) t -> p ko t", p=128)
                    [:, :, ch * CH:(ch + 1) * CH])
                _ln_chunk(nc, lnp, xc, xn, ch * CH, lnc)
            for ch in range(QN // CH):
                xc = xchunk.tile([128, NCT, CH], bf16, tag="xc")
                nc.sync.dma_start(
                    out=xc[:],
                    in_=d["xqT"].ap().rearrange("(ko p) t -> p ko t", p=128)
                    [:, :, ch * CH:(ch + 1) * CH])
                _ln_chunk(nc, lnp, xc, xnq, ch * CH, lnc)

            # --- Q^T --- (weights streamed in column halves)
            for wh in range(2):
                wsb = wstream.tile([128, NCT, CH], bf16, tag="w")
                nc.sync.dma_start(
                    out=wsb[:],
                    in_=d["wq_r"].ap().rearrange("(ko p) n -> p ko n", p=128)
                    [:, :, wh * CH:(wh + 1) * CH])
                for ml in range(4):
                    m = 4 * wh + ml
                    for qc in range(QN // CH):
                        sl = slice(qc * CH, (qc + 1) * CH)
                        ps = mm_ps.tile([128, CH], f32, tag="mm")
                        for ct in range(NCT):
                            nc.tensor.matmul(
                                ps[:], wsb[:, ct, ml * 128:(ml + 1) * 128],
                                xnq[:, ct, sl],
                                start=(ct == 0), stop=(ct == NCT - 1))
                        nc.vector.tensor_scalar(out=qT[:, m, sl], in0=ps[:],
                                                scalar1=qb_sb[:, m:m + 1],
                                                scalar2=None, op0=ALU.add)
            # --- K^T ---
            for wh in range(2):
                wsb = wstream.tile([128, NCT, CH], bf16, tag="w")
                nc.sync.dma_start(
                    out=wsb[:],
                    in_=d["wk_r"].ap().rearrange("(ko p) n -> p ko n", p=128)
                    [:, :, wh * CH:(wh + 1) * CH])
                for ml in range(4):
                    m = 4 * wh + ml
                    for sc in range(T // CH):
                        sl = slice(sc * CH, (sc + 1) * CH)
                        ps = mm_ps.tile([128, CH], f32, tag="mm")
                        for ct in range(NCT):
                            nc.tensor.matmul(
                                ps[:], wsb[:, ct, ml * 128:(ml + 1) * 128],
                                xn[:, ct, sl],
                                start=(ct == 0), stop=(ct == NCT - 1))
                        nc.vector.tensor_scalar(out=kt[:, m, sl], in0=ps[:],
                                                scalar1=kb_sb[:, m:m + 1],
                                                scalar2=None, op0=ALU.add)
            # --- V ---
            for half in range(2):
                wsb = wstream.tile([128, NCT, CH], bf16, tag="w")
                nc.sync.dma_start(
                    out=wsb[:],
                    in_=d["wv_r"].ap().rearrange("(ko p) n -> p ko n", p=128)
                    [:, :, half * CH:(half + 1) * CH])
                for st in range(NST):
                    ps = mm_ps.tile([128, CH], f32, tag="mm")
                    for ct in range(NCT):
                        nc.tensor.matmul(ps[:],
                                         xn[:, ct, st * 128:(st + 1) * 128],
                                         wsb[:, ct, :],
                                         start=(ct == 0), stop=(ct == NCT - 1))
                    psv = ps[:].rearrange("p (a x) -> p a x", a=4)
                    vbv = vb_sb[:].rearrange("p (hh a x) -> p hh a x", hh=2, a=4)
                    for b in range(2):
                        nc.vector.scalar_tensor_tensor(
                            out=v[:, 4 * half:4 * half + 4, st, b, 0:64],
                            in0=psv[:, :, b * 64:(b + 1) * 64],
                            scalar=0.0,
                            in1=vbv[:, half, :, b * 64:(b + 1) * 64],
                            op0=ALU.add, op1=ALU.add)

        # ---------------- phase B: attention + proj + LN2 + FFN ----------
        with ExitStack() as pB:
            bigB = pB.enter_context(tc.tile_pool(name="bigB", bufs=1))
            attnT = bigB.tile([128, NCT, QN], bf16)
            x2 = bigB.tile([128, NCT, QN], bf16)
            xn2 = bigB.tile([128, NCT, QN], bf16)
            hT = bigB.tile([128, NFT, CH], bf16)
            weip = pB.enter_context(tc.tile_pool(name="weip", bufs=2))
            maskp = pB.enter_context(tc.tile_pool(name="maskp", bufs=2))
            normp = pB.enter_context(tc.tile_pool(name="normp", bufs=2))
            w12p = pB.enter_context(tc.tile_pool(name="w12p", bufs=2))
            otp = pB.enter_context(tc.tile_pool(name="otp", bufs=1))
            sq2 = pB.enter_context(tc.tile_pool(name="lnsq2", bufs=1))
            rows2 = pB.enter_context(tc.tile_pool(name="lnrows2", bufs=1))
            absb2 = pB.enter_context(tc.tile_pool(name="lnabsb2", bufs=1))
            sc_ps = pB.enter_context(
                tc.tile_pool(name="scps", bufs=1, space="PSUM"))
            av_ps = pB.enter_context(
                tc.tile_pool(name="avps", bufs=1, space="PSUM"))
            mm2_ps = pB.enter_context(
                tc.tile_pool(name="mm2ps", bufs=2, space="PSUM"))
            ln2p = (sq2, rows2, absb2, mm2_ps, mm2_ps)

            # residual base: x2 starts as x (q columns)
            nc.sync.dma_start(
                out=x2[:],
                in_=d["xqT"].ap().rearrange("(ko p) t -> p ko t", p=128))

            def attention(qc):
                nst = NS[qc]
                qsl = slice(qc * CH, (qc + 1) * CH)
                for hp in range(NCT):
                    psAB = av_ps.tile([128, 2 * CH], f32, tag="av")
                    pend = None
                    for st in range(nst):
                        ssl = slice(st * 128, (st + 1) * 128)
                        scAB = sc_ps.tile([128, 2 * CH], f32, tag="sc",
                                          bufs=2)
                        nc.tensor.matmul(scAB[:, 0:CH], kt[0:64, hp, ssl],
                                         qT[0:64, hp, qsl],
                                         start=True, stop=True,
                                         tile_position=(0, 0))
                        nc.tensor.matmul(scAB[:, CH:2 * CH],
                                         kt[64:128, hp, ssl],
                                         qT[64:128, hp, qsl],
                                         start=True, stop=True,
                                         tile_position=(64, 0))
                        weiAB = weip.tile([128, 2 * CH], bf16, tag="wei",
                                          bufs=3)
                        nc.scalar.activation(out=weiAB[:], in_=scAB[:],
                                             func=AF.Exp)
                        if qc == 0 or st >= NS[0]:
                            mk = maskp.tile([128, CH], bf16, tag="mk")
                            nc.vector.tensor_scalar(
                                out=mk[:], in0=t_rep[:, qsl],
                                scalar1=s_iota[:, st:st + 1], scalar2=None,
                                op0=ALU.is_ge)
                            nc.vector.tensor_mul(weiAB[:, 0:CH],
                                                 weiAB[:, 0:CH], mk[:])
                            nc.vector.tensor_mul(weiAB[:, CH:2 * CH],
                                                 weiAB[:, CH:2 * CH], mk[:])
                        if pend is not None:
                            pst, pwei = pend
                            nc.tensor.matmul(psAB[0:65, 0:CH],
                                             v[:, hp, pst, 0, :],
                                             pwei[:, 0:CH],
                                             start=(pst == 0), stop=False)
                            nc.tensor.matmul(psAB[0:65, CH:2 * CH],
                                             v[:, hp, pst, 1, :],
                                             pwei[:, CH:2 * CH],
                                             start=(pst == 0), stop=False)
                        pend = (st, weiAB)
                    pst, pwei = pend
                    nc.tensor.matmul(psAB[0:65, 0:CH], v[:, hp, pst, 0, :],
                                     pwei[:, 0:CH],
                                     start=(pst == 0), stop=True)
                    nc.tensor.matmul(psAB[0:65, CH:2 * CH],
                                     v[:, hp, pst, 1, :],
                                     pwei[:, CH:2 * CH],
                                     start=(pst == 0), stop=True)
                    lnAB = normp.tile([65, 2 * CH], f32, tag="lnAB", bufs=1)
                    nc.scalar.activation(out=lnAB[64:65, :],
                                         in_=psAB[64:65, :], func=AF.Ln)
                    recAB = normp.tile([65, 2 * CH], bf16, tag="recAB",
                                       bufs=1)
                    nc.scalar.activation(out=recAB[64:65, :],
                                         in_=lnAB[64:65, :],
                                         func=AF.Exp, scale=scm1[64:65, :])
                    repAB = sc_ps.tile([128, 2 * CH], f32, tag="sc", bufs=2)
                    nc.tensor.matmul(repAB[:, 0:CH], ones_all[64:65, :],
                                     recAB[64:65, 0:CH],
                                     start=True, stop=True)
                    nc.tensor.matmul(repAB[:, CH:2 * CH], ones_all[64:65, :],
                                     recAB[64:65, CH:2 * CH],
                                     start=True, stop=True)
                    rsbAB = normp.tile([128, 2 * CH], f32, tag="rsbAB",
                                       bufs=1)
                    nc.vector.tensor_copy(rsbAB[:], repAB[:])
                    nc.vector.tensor_mul(attnT[0:64, hp, qsl],
                                         psAB[0:64, 0:CH], rsbAB[0:64, 0:CH])
                    tmpB = normp.tile([64, CH], bf16, tag="tmpB")
                    nc.vector.tensor_mul(tmpB[:], psAB[0:64, CH:2 * CH],
                                         rsbAB[0:64, CH:2 * CH])
                    nc.sync.dma_start(out=attnT[64:128, hp, qsl], in_=tmpB[:])

            def proj(qc):
                qsl = slice(qc * CH, (qc + 1) * CH)
                for ct in range(NCT):
                    wp = w12p.tile([128, NCT, 128], bf16, tag="w1")
                    nc.sync.dma_start(out=wp[:], in_=d["wp_t"].ap()[ct])
                    ps = mm2_ps.tile([128, CH], f32, tag="mm")
                    for m in range(NCT):
                        nc.tensor.matmul(ps[:], wp[:, m, :], attnT[:, m, qsl],
                                         start=(m == 0), stop=(m == NCT - 1))
                    nc.vector.scalar_tensor_tensor(
                        out=x2[:, ct, qsl], in0=ps[:],
                        scalar=bp_sb[:, ct:ct + 1], in1=x2[:, ct, qsl],
                        op0=ALU.add, op1=ALU.add)

            def ffn(qc):
                qsl = slice(qc * CH, (qc + 1) * CH)
                for ft in range(NFT):
                    w1s = w12p.tile([128, NCT, 128], bf16, tag="w1")
                    nc.sync.dma_start(out=w1s[:], in_=d["w1_t"].ap()[ft])
                    ps = mm2_ps.tile([128, CH], f32, tag="mm")
                    for ct in range(NCT):
                        nc.tensor.matmul(ps[:], w1s[:, ct, :], xn2[:, ct, qsl],
                                         start=(ct == 0), stop=(ct == NCT - 1))
                    nc.vector.tensor_scalar(out=hT[:, ft, :], in0=ps[:],
                                            scalar1=b1_sb[:, ft:ft + 1],
                                            scalar2=0.0, op0=ALU.add,
                                            op1=ALU.max)
                for ct in range(NCT):
                    ps = mm2_ps.tile([128, CH], f32, tag="mm")
                    for g in range(4):
                        w2s = w12p.tile([128, 8, 128], bf16, tag="w2")
                        nc.sync.dma_start(
                            out=w2s[:],
                            in_=d["w2_t"].ap()[ct][:, g * 8:(g + 1) * 8, :])
                        for f in range(8):
                            ft = g * 8 + f
                            nc.tensor.matmul(ps[:], w2s[:, f, :], hT[:, ft, :],
                                             start=(ft == 0),
                                             stop=(ft == NFT - 1))
                    ot = otp.tile([128, CH], bf16, tag="ot")
                    nc.vector.scalar_tensor_tensor(
                        out=ot[:], in0=ps[:], scalar=b2_sb[:, ct:ct + 1],
                        in1=x2[:, ct, qsl], op0=ALU.add, op1=ALU.add)
                    nc.sync.dma_start(
                        out=d["outT"].ap().rearrange(
                            "(ko p) t -> p ko t", p=128)[:, ct, qsl],
                        in_=ot[:])

            attention(0)
            proj(0)
            _ln_chunk2(nc, ln2p, x2, xn2, 0, lnc)
            attention(1)
            ffn(0)
            proj(1)
            _ln_chunk2(nc, ln2p, x2, xn2, CH, lnc)
            ffn(1)


def _ln_chunk(nc, pools, xc, dst_sb, col0, consts):
    """LN over one CH-column chunk; xc is [128, NCT, CH] (chunk of src),
    dst_sb is the full [128, NCT, *] output tile."""
    sqp, rows, absb, stat_ps, rep_ps = pools
    ones_lhs, ones_row, eps, scmh = consts
    sl = slice(col0, col0 + CH)
    ps_sum = stat_ps.tile([1, CH], f32, tag="ssum")
    ps_sq = stat_ps.tile([1, CH], f32, tag="ssq")
    for ct in range(NCT):
        sq = sqp.tile([128, CH], bf16, tag="ln_sq")
        nc.vector.tensor_mul(sq[:], xc[:, ct, :], xc[:, ct, :])
        nc.tensor.matmul(ps_sum[:], ones_lhs[:], xc[:, ct, :],
                         start=(ct == 0), stop=(ct == NCT - 1))
        nc.tensor.matmul(ps_sq[:], ones_lhs[:], sq[:],
                         start=(ct == 0), stop=(ct == NCT - 1))
    mean = rows.tile([1, CH], f32, tag="ln_mean")
    nc.vector.tensor_scalar_mul(mean[:], ps_sum[:], 1.0 / C)
    var = rows.tile([1, CH], f32, tag="ln_var")
    nc.vector.tensor_mul(var[:], mean[:], mean[:])
    nc.vector.scalar_tensor_tensor(out=var[:], in0=ps_sq[:], scalar=1.0 / C,
                                   in1=var[:], op0=ALU.mult, op1=ALU.subtract)
    lnv = rows.tile([1, CH], f32, tag="ln_lnv")
    nc.scalar.activation(out=lnv[:], in_=var[:], func=AF.Ln,
                         bias=eps[0:1, :], scale=1.0)
    arow = rows.tile([1, CH], bf16, tag="ln_a")
    nc.scalar.activation(out=arow[:], in_=lnv[:], func=AF.Exp,
                         scale=scmh[0:1, :])
    brow = rows.tile([1, CH], bf16, tag="ln_b")
    nc.vector.scalar_tensor_tensor(out=brow[:], in0=mean[:], scalar=-1.0,
                                   in1=arow[:], op0=ALU.mult, op1=ALU.mult)
    ps_a = rep_ps.tile([128, CH], f32, tag="ra")
    ps_b = rep_ps.tile([128, CH], f32, tag="rb")
    nc.tensor.matmul(ps_a[:], ones_row[:], arow[:], start=True, stop=True)
    nc.tensor.matmul(ps_b[:], ones_row[:], brow[:], start=True, stop=True)
    a_sb = absb.tile([128, CH], bf16, tag="ln_asb")
    b_sb = absb.tile([128, CH], bf16, tag="ln_bsb")
    nc.vector.tensor_copy(a_sb[:], ps_a[:])
    nc.vector.tensor_copy(b_sb[:], ps_b[:])
    for ct in range(NCT):
        nc.vector.tensor_mul(dst_sb[:, ct, sl], xc[:, ct, :], a_sb[:])
        nc.vector.tensor_add(dst_sb[:, ct, sl], dst_sb[:, ct, sl], b_sb[:])


def _ln_chunk2(nc, pools, src_sb, dst_sb, col0, consts):
    """LN over one CH-column chunk of a resident [128, NCT, QN] tile."""
    sqp, rows, absb, stat_ps, rep_ps = pools
    ones_lhs, ones_row, eps, scmh = consts
    sl = slice(col0, col0 + CH)
    ps_sum = stat_ps.tile([1, CH], f32, tag="mm")
    ps_sq = stat_ps.tile([1, CH], f32, tag="mm")
    for ct in range(NCT):
        sq = sqp.tile([128, CH], bf16, tag="ln_sq2")
        nc.vector.tensor_mul(sq[:], src_sb[:, ct, sl], src_sb[:, ct, sl])
        nc.tensor.matmul(ps_sum[:], ones_lhs[:], src_sb[:, ct, sl],
                         start=(ct == 0), stop=(ct == NCT - 1))
        nc.tensor.matmul(ps_sq[:], ones_lhs[:], sq[:],
                         start=(ct == 0), stop=(ct == NCT - 1))
    mean = rows.tile([1, CH], f32, tag="ln_mean2")
    nc.vector.tensor_scalar_mul(mean[:], ps_sum[:], 1.0 / C)
    var = rows.tile([1, CH], f32, tag="ln_var2")
    nc.vector.tensor_mul(var[:], mean[:], mean[:])
    nc.vector.scalar_tensor_tensor(out=var[:], in0=ps_sq[:], scalar=1.0 / C,
                                   in1=var[:], op0=ALU.mult, op1=ALU.subtract)
    lnv = rows.tile([1, CH], f32, tag="ln_lnv2")
    nc.scalar.activation(out=lnv[:], in_=var[:], func=AF.Ln,
                         bias=eps[0:1, :], scale=1.0)
    arow = rows.tile([1, CH], bf16, tag="ln_a2")
    nc.scalar.activation(out=arow[:], in_=lnv[:], func=AF.Exp,
                         scale=scmh[0:1, :])
    brow = rows.tile([1, CH], bf16, tag="ln_b2")
    nc.vector.scalar_tensor_tensor(out=brow[:], in0=mean[:], scalar=-1.0,
                                   in1=arow[:], op0=ALU.mult, op1=ALU.mult)
    ps_a = rep_ps.tile([128, CH], f32, tag="mm")
    nc.tensor.matmul(ps_a[:], ones_row[:], arow[:], start=True, stop=True)
    a_sb = absb.tile([128, CH], bf16, tag="ln_asb2")
    nc.vector.tensor_copy(a_sb[:], ps_a[:])
    ps_b = rep_ps.tile([128, CH], f32, tag="mm")
    nc.tensor.matmul(ps_b[:], ones_row[:], brow[:], start=True, stop=True)
    b_sb = absb.tile([128, CH], bf16, tag="ln_bsb2")
    nc.vector.tensor_copy(b_sb[:], ps_b[:])
    for ct in range(NCT):
        nc.vector.tensor_mul(dst_sb[:, ct, sl], src_sb[:, ct, sl], a_sb[:])
        nc.vector.tensor_add(dst_sb[:, ct, sl], dst_sb[:, ct, sl], b_sb[:])


def make_host_inputs(inputs):
    """Build per-core in_maps from the full problem inputs."""
    import ml_dtypes
    bfl = ml_dtypes.bfloat16

    x = np.asarray(inputs["x"], np.float32)
    wq = np.asarray(inputs["wq"], np.float32)
    wk = np.asarray(inputs["wk"], np.float32)
    wv = np.asarray(inputs["wv"], np.float32)
    w_proj = np.asarray(inputs["w_proj"], np.float32)
    b_proj = np.asarray(inputs["b_proj"], np.float32)
    w1 = np.asarray(inputs["w1"], np.float32)
    b1 = np.asarray(inputs["b1"], np.float32)
    w2 = np.asarray(inputs["w2"], np.float32)
    b2 = np.asarray(inputs["b2"], np.float32)
    g1 = np.asarray(inputs["g1"], np.float32)
    be1 = np.asarray(inputs["be1"], np.float32)
    g2 = np.asarray(inputs["g2"], np.float32)
    be2 = np.asarray(inputs["be2"], np.float32)

    scale = np.float32(C ** -0.5)
    wq_eff = wq * g1[None, :, None]
    wk_eff = wk * g1[None, :, None]
    wv_eff = wv * g1[None, :, None]
    wq_r = np.ascontiguousarray(
        wq_eff.transpose(1, 0, 2).reshape(C, C) * scale).astype(bfl)
    wk_r = np.ascontiguousarray(
        wk_eff.transpose(1, 0, 2).reshape(C, C)).astype(bfl)
    wv_r = np.ascontiguousarray(
        wv_eff.transpose(1, 0, 2).reshape(C, C)).astype(bfl)
    qb = (np.einsum("c,hcd->hd", be1, wq_eff).reshape(C) * scale)
    kb = np.einsum("c,hcd->hd", be1, wk_eff).reshape(C)
    vb = np.einsum("c,hcd->hd", be1, wv_eff).reshape(C)
    wp_t = np.ascontiguousarray(
        w_proj.reshape(NCT, 128, NCT, 128).transpose(2, 1, 0, 3)).astype(bfl)
    w1_eff = w1 * g2[:, None]
    b1_eff = b1 + be2 @ w1
    w1_t = np.ascontiguousarray(
        w1_eff.reshape(NCT, 128, NFT, 128).transpose(2, 1, 0, 3)).astype(bfl)
    w2_t = np.ascontiguousarray(
        w2.reshape(NFT, 128, NCT, 128).transpose(2, 1, 0, 3)).astype(bfl)

    shared = {
        "wq_r": wq_r, "wk_r": wk_r, "wv_r": wv_r,
        "wp_t": wp_t, "w1_t": w1_t, "w2_t": w2_t,
        "qb_in": np.ascontiguousarray(qb.reshape(NCT, 128).T),
        "kb_in": np.ascontiguousarray(kb.reshape(NCT, 128).T),
        "vb_rep": np.broadcast_to(vb[None, :], (128, C)).astype(bfl).copy(),
        "bproj_in": np.ascontiguousarray(b_proj.reshape(NCT, 128).T),
        "b1_in": np.ascontiguousarray(b1_eff.reshape(NFT, 128).T),
        "b2_in": np.ascontiguousarray(b2.reshape(NCT, 128).T),
        "ones_lhs": np.ones((128, 1), bfl),
        "ones_row": np.ones((1, 128), bfl),
        "ones_all": np.ones((128, 128), bfl),
        "s_iota": np.ascontiguousarray(
            np.arange(T, dtype=np.float32).reshape(NST, 128).T),
    }

    in_maps = []
    qrows_all = []
    xT_cache = {}
    for core in range(8):
        b = core // 2
        j = core % 2
        qrows = np.r_[512 * j:512 * j + 512, 1024 + 512 * j:1536 + 512 * j]
        qrows_all.append((b, qrows))
        if b not in xT_cache:
            xT_cache[b] = np.ascontiguousarray(x[b].T).astype(bfl)
        m = dict(shared)
        m["xT"] = xT_cache[b]
        m["xqT"] = np.ascontiguousarray(x[b][qrows].T).astype(bfl)
        m["t_rep"] = np.broadcast_to(
            qrows.astype(np.float32)[None, :], (128, QN)).copy()
        in_maps.append(m)
    return in_maps, qrows_all


def gather_output(results, qrows_all):
    out = np.zeros((B, T, C), np.float32)
    for core, (b, qrows) in enumerate(qrows_all):
        out[b, qrows, :] = np.asarray(results[core]["outT"],
                                      dtype=np.float32).T
    return out


_NC_CACHE = {}


def kernel(**inputs):
    from concourse.bass_utils import run_bass_kernel_spmd

    if "nc" not in _NC_CACHE:
        nc = build_program()
        split_excess_waits(nc)
        _NC_CACHE["nc"] = nc
    nc = _NC_CACHE["nc"]
    in_maps, qrows_all = make_host_inputs(inputs)
    res = run_bass_kernel_spmd(nc, in_maps, core_ids=list(range(8)))
    return gather_output(res.results, qrows_all)
